# revision 8
# baseline (speedup 1.0000x reference)
"""Trainium2 Bass kernel for KCMemory.causal_sequence (scatter_memory).

Sequence-parallel chunked scan (RetNet-style):
  - T=8192 rows sharded contiguously over 8 NeuronCores (1024 each).
  - Per core, 8 chunks of C=128. Per chunk the decayed cumulative state
    M_chunk = A^T @ S + alpha^(i+1) x N_prev is ONE PE accumulation group
    per 512-wide PSUM slice (A is a precomputed 128x128 decay matrix,
    S[s, r*128+v] = phi[s,r]*V[s,v] built by 64 DVE tensor_scalar ops).
  - v_hat via Gram-matrix trick: vh = (A .* (phi phi^T))^T @ V + (phi*a) @ N.
  - Cross-core carry: each core computes a decayed summary F_c (one matmul),
    AllGather (32KB), combine with per-core weights to get the start state.
"""
import sys
import numpy as np

sys.path.insert(0, "/opt/trn_rl_repo")
sys.path.insert(0, "/opt/trn_rl_repo/concourse")

LAM = 0.001
ALPHA = 1.0 - LAM
T, DK, R, DV = 8192, 1024, 64, 128
NC_CORES = 8
TC = T // NC_CORES   # 1024
C = 128              # chunk
NCH = TC // C        # 8
NRV = R * DV         # 8192
NSL = NRV // 512     # 16 psum slices


def _consts():
    i = np.arange(C, dtype=np.float64)
    s = i
    A = LAM * ALPHA ** (i[None, :] - s[:, None])
    A = np.where(i[None, :] >= s[:, None], A, 0.0).astype(np.float32)   # (128,128) lhsT
    alpha_row = (ALPHA ** (i + 1)).astype(np.float32)[None, :]           # (1,128)
    alpha_bc = np.ascontiguousarray(np.broadcast_to(alpha_row, (R, C))).astype(np.float32)
    p = np.arange(128, dtype=np.float64)
    tb = np.arange(8, dtype=np.float64)
    wvec = (LAM * ALPHA ** (1023.0 - (tb[None, :] * 128 + p[:, None]))).astype(np.float32)
    wcomb = np.zeros((NC_CORES, 9), dtype=np.float32)
    for c in range(NC_CORES):
        for b in range(c):
            wcomb[c, b] = ALPHA ** (1024.0 * (c - 1 - b))
        wcomb[c, 8] = ALPHA ** (1024.0 * c)
    ident = np.eye(128, dtype=np.float32)
    return A, alpha_row, alpha_bc, wvec, wcomb, ident


_CACHE = {}


def build():
    if "nc" in _CACHE:
        return _CACHE["nc"]
    import concourse.bass as bass
    import concourse.tile as tile
    from concourse import bacc, mybir

    f32 = mybir.dt.float32
    nc = bacc.Bacc("TRN2", target_bir_lowering=False, debug=False,
                   num_devices=NC_CORES)

    keys_in = nc.dram_tensor("keys_sh", [TC, DK], f32, kind="ExternalInput")
    vals_in = nc.dram_tensor("vals_sh", [TC, DV], f32, kind="ExternalInput")
    w_in = nc.dram_tensor("w_phi", [R, DK], f32, kind="ExternalInput")
    b_in = nc.dram_tensor("b_col", [R, 1], f32, kind="ExternalInput")
    m0_in = nc.dram_tensor("m0", [R, DV], f32, kind="ExternalInput")
    a_in = nc.dram_tensor("a_mat", [C, C], f32, kind="ExternalInput")
    arow_in = nc.dram_tensor("a_row", [1, C], f32, kind="ExternalInput")
    abc_in = nc.dram_tensor("a_bc", [R, C], f32, kind="ExternalInput")
    wvec_in = nc.dram_tensor("wvec", [128, 8], f32, kind="ExternalInput")
    wcomb_in = nc.dram_tensor("wcomb", [R, 9], f32, kind="ExternalInput")
    id_in = nc.dram_tensor("ident", [128, 128], f32, kind="ExternalInput")

    m_out = nc.dram_tensor("m_out", [TC, NRV], f32, kind="ExternalOutput")
    v_out = nc.dram_tensor("v_out", [TC, DV], f32, kind="ExternalOutput")

    RELU = mybir.ActivationFunctionType.Relu
    COPY = mybir.ActivationFunctionType.Copy

    with tile.TileContext(nc) as tc:
        with (
            tc.tile_pool(name="const", bufs=1) as cpool,
            tc.tile_pool(name="vpool", bufs=1) as vpool,
            tc.tile_pool(name="psA", bufs=4, space="PSUM") as psA,
            tc.tile_pool(name="psB", bufs=3, space="PSUM") as psB,
            tc.tile_pool(name="dram", bufs=1, space="DRAM") as dpool,
            tc.tile_pool(name="small", bufs=2) as spool_sm,
        ):
            # ---- constants to SBUF ----
            A_sb = cpool.tile([C, C], f32, tag="A")
            nc.sync.dma_start(A_sb[:], a_in[:])
            arow_sb = cpool.tile([1, C], f32, tag="arow")
            nc.sync.dma_start(arow_sb[:], arow_in[:])
            abc_sb = cpool.tile([R, C], f32, tag="abc")
            nc.sync.dma_start(abc_sb[:], abc_in[:])
            wvec_sb = cpool.tile([128, 8], f32, tag="wvec")
            nc.sync.dma_start(wvec_sb[:], wvec_in[:])
            wcomb_sb = cpool.tile([R, 9], f32, tag="wcomb")
            nc.sync.dma_start(wcomb_sb[:], wcomb_in[:])
            id_sb = cpool.tile([128, 128], f32, tag="ident")
            nc.sync.dma_start(id_sb[:], id_in[:])
            b_sb = cpool.tile([R, 1], f32, tag="bcol")
            nc.sync.dma_start(b_sb[:], b_in[:])
            m0_sb = cpool.tile([R, DV], f32, tag="m0")
            nc.sync.dma_start(m0_sb[:], m0_in[:])

            # V as (128, 8*128): block tb holds rows tb*128..tb*128+127
            V_sb = vpool.tile([128, NCH * DV], f32, tag="V")
            nc.sync.dma_start(V_sb[:].rearrange("p (b v) -> p b v", v=DV),
                              vals_in.rearrange("(b p) v -> p b v", p=128))

            phiT_sb = cpool.tile([R, TC], f32, tag="phiT")       # (64, 1024)
            phi_sb = cpool.tile([128, NCH * R], f32, tag="phi")  # (128, 512)
            phiw_sb = cpool.tile([128, NCH * R], f32, tag="phiw")
            Wt_sb = cpool.tile([128, 8 * R], f32, tag="Wt")      # (128, 512)
            fcall_sb = cpool.tile([R, 8 * DV], f32, tag="fcall")  # (64, 1024)
            fc_sb = cpool.tile([R, DV], f32, tag="fc")

            # ---- phase 1: transposes + phi ----
            with (
                tc.tile_pool(name="keys", bufs=3) as kpool,
                tc.tile_pool(name="keysT", bufs=1) as ktpool,
                tc.tile_pool(name="wtmp", bufs=1) as wpool,
            ):
                # W^T: (64,1024) -> 8 blocks (128,64) packed in one psum bank
                W_sb = wpool.tile([R, DK], f32, tag="Wsb")
                nc.sync.dma_start(W_sb[:], w_in[:])
                psW = psB.tile([128, 512], f32, tag="psB")
                for kt in range(8):
                    nc.tensor.transpose(
                        psW[:, kt * 64:(kt + 1) * 64],
                        W_sb[:, kt * 128:(kt + 1) * 128],
                        id_sb[0:64, 0:64])
                nc.scalar.activation(Wt_sb[:], psW[:], COPY)

                keysT = ktpool.tile([128, NRV], f32, tag="keysT")  # (128, 8192)
                for tt in range(8):
                    ktile = kpool.tile([128, DK], f32, tag="keys")
                    nc.sync.dma_start(ktile[:], keys_in[tt * 128:(tt + 1) * 128, :])
                    for g in range(2):  # two groups of 4 k-tiles
                        psT = psA.tile([128, 512], f32, tag="psA")
                        for q in range(4):
                            kt = g * 4 + q
                            nc.tensor.transpose(
                                psT[:, q * 128:(q + 1) * 128],
                                ktile[:, kt * 128:(kt + 1) * 128],
                                id_sb[:, :])
                        # scatter 4 blocks into keysT at [kt*1024 + tt*128]
                        dst = keysT[:, :].rearrange("p (k t) -> p k t", t=1024)
                        nc.scalar.activation(
                            dst[:, g * 4:(g + 1) * 4, tt * 128:(tt + 1) * 128],
                            psT[:].rearrange("p (k t) -> p k t", t=128), COPY)

                # phi^T = relu(W keys^T + b): accumulate 8 k-tiles
                for half in range(2):
                    psP = psB.tile([R, 512], f32, tag="psB")
                    for kt in range(8):
                        nc.tensor.matmul(
                            psP[:],
                            Wt_sb[:, kt * 64:(kt + 1) * 64],
                            keysT[:, kt * 1024 + half * 512: kt * 1024 + (half + 1) * 512],
                            start=(kt == 0), stop=(kt == 7))
                    nc.scalar.activation(
                        phiT_sb[:, half * 512:(half + 1) * 512], psP[:],
                        RELU, bias=b_sb[:, 0:1])

            # phi (time-on-partitions): 8 transposed blocks packed in one bank
            psF2 = psB.tile([128, 512], f32, tag="psB")
            for j in range(8):
                nc.tensor.transpose(
                    psF2[:, j * 64:(j + 1) * 64],
                    phiT_sb[:, j * 128:(j + 1) * 128],
                    id_sb[0:64, 0:64])
            nc.scalar.activation(phi_sb[:], psF2[:], COPY)

            # ---- F_c summary + AllGather + N_start ----
            for tb_ in range(8):
                nc.vector.tensor_scalar_mul(
                    phiw_sb[:, tb_ * 64:(tb_ + 1) * 64],
                    phi_sb[:, tb_ * 64:(tb_ + 1) * 64],
                    wvec_sb[:, tb_:tb_ + 1])
            psFc = psB.tile([R, DV], f32, tag="psB")
            for tb_ in range(8):
                nc.tensor.matmul(
                    psFc[:],
                    phiw_sb[:, tb_ * 64:(tb_ + 1) * 64],
                    V_sb[:, tb_ * 128:(tb_ + 1) * 128],
                    start=(tb_ == 0), stop=(tb_ == 7))
            nc.scalar.activation(fc_sb[:], psFc[:], COPY)

            fc_in_d = dpool.tile([R, DV], f32, tag="fcin")
            fc_all_d = dpool.tile([NC_CORES * R, DV], f32, tag="fcall")
            nc.sync.dma_start(fc_in_d[:], fc_sb[:])
            nc.gpsimd.collective_compute(
                "AllGather",
                bass.mybir.AluOpType.bypass,
                replica_groups=[list(range(NC_CORES))],
                ins=[fc_in_d.opt()],
                outs=[fc_all_d.opt()],
            )
            nc.sync.dma_start(
                fcall_sb[:].rearrange("r (b v) -> r b v", v=DV),
                fc_all_d[:].rearrange("(b r) v -> r b v", r=R))

            with (
                tc.tile_pool(name="spool", bufs=2) as spool,
                tc.tile_pool(name="opool", bufs=2) as opool,
                tc.tile_pool(name="vhpool", bufs=2) as vhpool,
                tc.tile_pool(name="nrow", bufs=1) as nrpool,
                tc.tile_pool(name="nmat", bufs=2) as nmpool,
                tc.tile_pool(name="gpool", bufs=2) as gpool,
            ):
                # N_start = wcomb[8]*M0 + sum_b wcomb[b]*F_b
                nmat0 = nmpool.tile([R, DV], f32, tag="nmat")
                nc.vector.tensor_scalar_mul(nmat0[:], m0_sb[:], wcomb_sb[:, 8:9])
                for b in range(8):
                    tmp = spool_sm.tile([R, DV], f32, tag="ntmp")
                    nc.vector.tensor_scalar_mul(
                        tmp[:], fcall_sb[:, b * DV:(b + 1) * DV],
                        wcomb_sb[:, b:b + 1])
                    nc.vector.tensor_add(nmat0[:], nmat0[:], tmp[:])
                nrow0 = nrpool.tile([1, NRV], f32, tag="nrow")
                nc.sync.dma_start(nrow0[0:1, :], nmat0[:])

                n_mat, n_row = nmat0, nrow0
                for j in range(NCH):
                    Vj = V_sb[:, j * DV:(j + 1) * DV]
                    # S build: 64 per-partition-scalar muls
                    S_t = spool.tile([128, NRV], f32, tag="S")
                    for r in range(R):
                        nc.vector.tensor_scalar_mul(
                            S_t[:, r * DV:(r + 1) * DV], Vj,
                            phi_sb[:, j * R + r: j * R + r + 1])
                    out_sb = opool.tile([128, NRV], f32, tag="out")
                    for sl in range(NSL):
                        ps = psA.tile([128, 512], f32, tag="psA")
                        nc.tensor.matmul(ps[:], A_sb[:],
                                         S_t[:, sl * 512:(sl + 1) * 512],
                                         start=True, stop=False)
                        nc.tensor.matmul(ps[:], arow_sb[0:1, :],
                                         n_row[0:1, sl * 512:(sl + 1) * 512],
                                         start=False, stop=True)
                        nc.scalar.activation(
                            out_sb[:, sl * 512:(sl + 1) * 512], ps[:], COPY)
                    nc.sync.dma_start(m_out[j * 128:(j + 1) * 128, :], out_sb[:])

                    # v_hat
                    phiTj = phiT_sb[:, j * C:(j + 1) * C]
                    psPm = psB.tile([C, C], f32, tag="psB")
                    nc.tensor.matmul(psPm[:], phiTj, phiTj, start=True, stop=True)
                    G_sb = gpool.tile([C, C], f32, tag="G")
                    nc.vector.tensor_mul(G_sb[:], A_sb[:], psPm[:])
                    phiTa = gpool.tile([R, C], f32, tag="phiTa")
                    nc.vector.tensor_mul(phiTa[:], phiTj, abc_sb[:])
                    psV = psB.tile([C, DV], f32, tag="psB")
                    nc.tensor.matmul(psV[:], G_sb[:], Vj, start=True, stop=False)
                    nc.tensor.matmul(psV[:], phiTa[:], n_mat[:],
                                     start=False, stop=True)
                    vh = vhpool.tile([C, DV], f32, tag="vh")
                    nc.scalar.activation(vh[:], psV[:], COPY)
                    nc.sync.dma_start(v_out[j * 128:(j + 1) * 128, :], vh[:])

                    if j + 1 < NCH:
                        n_row = nrpool.tile([1, NRV], f32, tag="nrow")
                        nc.sync.dma_start(n_row[0:1, :], out_sb[127:128, :])
                        n_mat = nmpool.tile([R, DV], f32, tag="nmat")
                        nc.sync.dma_start(
                            n_mat[:],
                            out_sb[127:128, :].rearrange("o (r v) -> o r v", v=DV))

    nc.compile()
    _CACHE["nc"] = nc
    return nc


def kernel(keys, values, W_phi, b_phi, M):
    from concourse.bass_utils import run_bass_kernel_spmd

    nc = build()
    A, alpha_row, alpha_bc, wvec, wcomb, ident = _consts()
    keys = np.ascontiguousarray(np.asarray(keys, dtype=np.float32))
    values = np.ascontiguousarray(np.asarray(values, dtype=np.float32))
    W_phi = np.ascontiguousarray(np.asarray(W_phi, dtype=np.float32))
    b_col = np.asarray(b_phi, dtype=np.float32).reshape(R, 1)
    m0 = np.ascontiguousarray(np.asarray(M, dtype=np.float32).reshape(R, DV))

    in_maps = []
    for c in range(NC_CORES):
        in_maps.append({
            "keys_sh": keys[c * TC:(c + 1) * TC],
            "vals_sh": values[c * TC:(c + 1) * TC],
            "w_phi": W_phi,
            "b_col": b_col,
            "m0": m0,
            "a_mat": A,
            "a_row": alpha_row,
            "a_bc": alpha_bc,
            "wvec": wvec,
            "wcomb": np.ascontiguousarray(
                np.broadcast_to(wcomb[c][None, :], (R, 9))).astype(np.float32),
            "ident": ident,
        })
    res = run_bass_kernel_spmd(nc, in_maps, list(range(NC_CORES))).results
    m_seq = np.concatenate(
        [res[c]["m_out"].reshape(TC, R, DV) for c in range(NC_CORES)], axis=0)
    v_hat = np.concatenate(
        [res[c]["v_out"] for c in range(NC_CORES)], axis=0)
    return m_seq, v_hat
